# revision 10
# baseline (speedup 1.0000x reference)
"""BrainGCN Trainium2 kernel: 2x GCNConv + 3 FC layers over a 100K-node,
1.6M-edge random graph, distributed over 8 NeuronCores.

Strategy (all FLOPs on x-dependent data run on device):
- Nodes (dst) sharded across 8 cores: core c owns dst nodes [c*12500, (c+1)*12500).
- Aggregate-first formulation: agg[d] = sum_e w_e * x[src_e], then (agg @ W).
- The symmetric norm w_e = dinv[src]*dinv[dst] is FACTORED OUT of the
  per-tile one-hot matrices: gather tables are pre-scaled by dinv[src] on
  device (per-core scale pass + AllGather), and dinv[dst] is applied once
  per 128-slot block in the epilogue. The one-hot "selection" matrix S is
  then pure 0/1: ONE single-op is_equal tensor_scalar per 128-edge tile
  (vs. a fused is_equal+mult with two scalar reads, ~2.8x slower), and the
  self-loop S collapses to a constant identity (no vector op at all).
- Both conv layers gather from tables in the SAME permuted row space
  [NROWS2, 128] (scaled-x table for L1, scaled-h1 for L2), so one stream
  (indices + slot metadata) serves both layers.
- Messages fetched with gpsimd dma_gather (one 256B descriptor per edge),
  calls of 1024 idxs round-robined over 4 SWDGE queues. int16 index limit
  -> gather split into 4 row buckets (<=30000 rows).
- Per core, dst nodes are packed into 98 blocks of 128 slots, vector-LPT
  balanced on per-bucket in-edge counts. Per-(bucket,block) tile counts are
  maxed over cores so ONE SPMD program serves all 8 cores (pad entries get
  slot=-1 => zero contribution).
- Segment-sum is a PE matmul: psum[feat, slot] += matmul(lhsT=Xg[128 edges,
  feat], rhs=S[128 edges, 128 slots]).
- Between the scale pass and conv1, and between conv1 and conv2, an
  AllGather shares the per-core scaled table shards.
- FC layers run per-block in feature-major form on the PE.

Host-side work is limited to graph-structure preprocessing (degrees, norms,
permutations, index/metadata arrays, row reordering/casting of x) and final
unpermutation.
"""

import os
import sys
import types

import numpy as np


def _install_ntff_hook():
    """Image's antenv lacks axon_hooks; shim it so trace=True can profile."""
    if "antenv.axon_hooks" in sys.modules:
        return
    mod = types.ModuleType("antenv.axon_hooks")
    mod._hook = None
    mod.set_axon_ntff_profile_hook = lambda h: setattr(mod, "_hook", h)
    mod.get_axon_ntff_profile_hook = lambda: mod._hook
    sys.modules["antenv.axon_hooks"] = mod
    try:
        import antenv
        antenv.axon_hooks = mod
        from trn_agent_boot.trn_boot import _ntff_profile_via_ctypes
        mod.set_axon_ntff_profile_hook(
            _ntff_profile_via_ctypes("/opt/axon/libaxon_pjrt.so")
        )
    except Exception:
        pass


_install_ntff_hook()

import ml_dtypes
import concourse.bacc as bacc
import concourse.bass as bass  # noqa: F401
import concourse.mybir as mybir
import concourse.tile as tile
from concourse.bass_utils import run_bass_kernel_spmd

# ---------------------------------------------------------------- constants
N = 100000
D_IN = 128
H1 = 64
NCORES = 8
SHARD = N // NCORES            # 12500
BLKN = 98                      # blocks of 128 slots per core
SLOTS = BLKN * 128             # 12544
NROWS2 = NCORES * SLOTS        # 100352 rows in the allgathered tables
NBUCK = 4
BSIZE = 30000                  # gather bucket size (int16 reach)
BASES = [0, BSIZE, 2 * BSIZE, 3 * BSIZE]
SIZES2 = [BSIZE, BSIZE, BSIZE, NROWS2 - 3 * BSIZE]
CHUNK_TILES = 8                # tiles per dma_gather call (<=128 ring descs)
QBLK = [25, 25, 24, 24]        # dst-block quarters (pipelined AllGathers)
QBLO = [0, 25, 50, 74, 98]     # cumulative block offsets
QSLOT = [q * 128 for q in QBLK]
QROWS = [NCORES * s for s in QSLOT]
Q_OF_BLOCK = sum(([g] * QBLK[g] for g in range(4)), [])

LAST_EXEC_TIME_NS = None       # filled when BASS_GCN_TRACE=1
LAST_RESULTS = None


# ------------------------------------------------------------- host planning
def _lpt_assign_vec(loads):
    """Pack nodes into BLKN blocks x 128 slots, balancing the per-bucket load
    vectors (sum-of-squares greedy, descending total load)."""
    n = loads.shape[0]
    order = np.argsort(-loads.sum(1), kind="stable")
    block_loads = np.zeros((BLKN, loads.shape[1]), np.float64)
    used = np.zeros(BLKN, np.int64)
    pos = np.empty(n, np.int64)
    for i in order:
        li = loads[i]
        cand = block_loads + li
        score = np.einsum("ij,ij->i", cand, cand)
        score[used >= 128] = np.inf
        b = int(np.argmin(score))
        pos[i] = b * 128 + used[b]
        block_loads[b] += li
        used[b] += 1
    return pos


def _bucket_of(rows):
    return np.minimum(rows // BSIZE, NBUCK - 1)


def _build_stream(bucket, lrow, slots, T, np_gdt, grouped):
    """Scatter edges into the uniform padded stream. Key order matches the
    emission order: (dst-quarter if grouped, bucket, block); capacity of
    (be, b) is 128*T[be][b] within b's own quarter. Prebuilds the one-hot
    S matrices: smat[128, P], column block t = S_t [edge-in-tile, slot]."""
    blk = slots // 128
    qarr = np.asarray(Q_OF_BLOCK, np.int64)
    g = qarr[blk] if grouped else np.zeros_like(blk)
    key = (g * NBUCK + bucket) * BLKN + blk

    ngroups = 4 if grouped else 1
    cap = np.zeros(ngroups * NBUCK * BLKN, np.int64)
    for gq in range(ngroups):
        for be in range(NBUCK):
            for b in range(BLKN):
                if (not grouped) or qarr[b] == gq:
                    cap[(gq * NBUCK + be) * BLKN + b] = T[be][b]
    P = 128 * int(cap.sum())
    dest_base = np.zeros(len(cap) + 1, np.int64)
    np.cumsum(128 * cap, out=dest_base[1:])

    order = np.lexsort((lrow, key))
    skey = key[order]
    counts = np.bincount(skey, minlength=len(cap))
    starts = np.zeros(len(cap) + 1, np.int64)
    np.cumsum(counts, out=starts[1:])
    rank = np.arange(len(order)) - starts[skey]
    dest = dest_base[skey] + rank

    out_lrow = np.zeros(P, np.int16)
    out_lrow[dest] = lrow[order]
    idx_wrapped = np.tile(out_lrow.reshape(-1, 16).T, (8, 1))  # [128, P//16]

    smat = np.zeros((128, P), np_gdt)
    scol = (dest // 128) * 128 + slots[order] % 128
    smat[dest % 128, scol] = 1.0
    return idx_wrapped, smat


S_FP8 = os.environ.get("BASS_GCN_SFP8", "1") == "1"


def _plan(src, dst, x, np_gdt):
    """Full host-side graph preprocessing."""
    deg = (np.bincount(dst, minlength=N) + 1.0).astype(np.float64)
    dinv = (1.0 / np.sqrt(deg)).astype(np.float32)

    core_of = dst // SHARD
    src_core = src // SHARD

    # LPT balance on total in-degree (bucket = src-slot quarter is
    # placement-dependent; approximate with the rank-1 quarter split).
    indeg = np.bincount(dst, minlength=N).astype(np.float64)
    qfrac = np.asarray(QBLK, np.float64) / BLKN
    loads = indeg[:, None] * qfrac[None, :]

    pos_local = np.empty(N, np.int64)
    node_of_pos = np.full((NCORES, SLOTS), -1, np.int64)
    for c in range(NCORES):
        nodes = np.arange(c * SHARD, (c + 1) * SHARD)
        p = _lpt_assign_vec(loads[nodes])
        pos_local[nodes] = p
        node_of_pos[c, p] = nodes

    qarr = np.asarray(Q_OF_BLOCK, np.int64)
    src_pos = pos_local[src]
    q_src = qarr[src_pos // 128]
    qslot = np.asarray(QSLOT, np.int64)
    qblo = np.asarray(QBLO, np.int64)
    lrow = src_core * qslot[q_src] + (src_pos - qblo[q_src] * 128)

    counts = np.zeros((NCORES, NBUCK, BLKN), np.int64)
    for c in range(NCORES):
        m = core_of == c
        blk = pos_local[dst[m]] // 128
        counts[c] = np.bincount(
            q_src[m] * BLKN + blk, minlength=NBUCK * BLKN
        ).reshape(NBUCK, BLKN)

    T = np.ceil(counts.max(axis=0) / 128).astype(np.int64)

    streams = []
    xperms = []
    dinvcols = []
    dinvmats = []
    for c in range(NCORES):
        m = core_of == c
        slots = pos_local[dst[m]]
        idx1, smat1 = _build_stream(q_src[m], lrow[m], slots, T, np_gdt, True)
        idx2, smat2 = _build_stream(q_src[m], lrow[m], slots, T, np_gdt, False)
        streams.append((idx1, smat1, idx2, smat2))

        xp = np.zeros((SLOTS, D_IN), np.float32)
        valid = node_of_pos[c] >= 0
        nodes = node_of_pos[c][valid]
        xp[valid] = x[nodes]
        dcol_flat = np.zeros(SLOTS, np.float32)
        dcol_flat[valid] = dinv[nodes]
        dcol = np.ascontiguousarray(dcol_flat.reshape(BLKN, 128).T)  # [128, BLKN]
        dmat = np.broadcast_to(
            dcol_flat.astype(ml_dtypes.bfloat16), (128, SLOTS)
        ).copy()                                                     # [128, SLOTS]
        xperms.append(xp)
        dinvcols.append(dcol)
        dinvmats.append(dmat)

    return streams, xperms, dinvcols, dinvmats, T, node_of_pos


# ------------------------------------------------------------ device program
def _emit_conv(nc, pools, cfg):
    """Emit one conv layer: gather + one-hot matmul runs + per-block SBUF acc
    + self-loop run + epilogue. cfg["groups"] = list of (block_list, cb):
    tiles are emitted (group, bucket, block)-ordered; cb() fires after the
    group's blocks (incl. epilogues) are emitted -- used to launch the
    pipelined quarter-AllGathers."""
    f32 = mybir.dt.float32
    gdt = cfg["gdt"]              # gather-table dtype (f32 or bf16)
    T = cfg["T"]
    DF = cfg["feat"]              # features used for matmul lhsT
    GE = cfg["gelem"]             # gather elem_size (table row elements)
    table = cfg["table"]          # fn(bucket) -> DRAM AP
    self_rows = cfg["self_rows"]  # fn(block) -> DRAM AP [128, DF]
    idx_dram = cfg["idx"]
    smat_dram = cfg["smat"]
    tag = cfg["tag"]
    sb, sp, ps_run = pools["sb"], pools["sp"], pools["ps_run"]
    identg_t = cfg["identg_t"]    # [128,128] sdt identity (self-loop S)

    cfg.setdefault("_q", 0)
    acc_tiles = {}
    last_beta = np.full(BLKN, -1, np.int64)
    for b in range(BLKN):
        nz = [be for be in range(NBUCK) if T[be][b] > 0]
        if nz:
            last_beta[b] = nz[-1]

    def self_run_and_epilogue(b):
        xs = sb.tile([128, DF], gdt, tag="xself")
        nc.scalar.dma_start(xs[:], self_rows(b))
        psum = ps_run.tile([DF, 128], f32, tag="runps")
        nc.tensor.matmul(psum[:], xs[:], identg_t[:], start=True, stop=True)
        if b not in acc_tiles:
            acc_tiles[b] = pools["accp"].tile(
                [DF, 128], f32, tag=f"acc{b}", name=f"acc{tag}_{b}"
            )
            nc.vector.tensor_copy(acc_tiles[b][:], psum[:])
        else:
            nc.vector.tensor_add(acc_tiles[b][:], acc_tiles[b][:], psum[:])
        cfg["epilogue"](b, acc_tiles[b])

    gi = 0
    cur_ps = None
    for blocks, group_cb in cfg["groups"]:
        for be in range(NBUCK):
            sub = [(b, t) for b in blocks for t in range(int(T[be][b]))]
            i = 0
            while i < len(sub):
                k = min(CHUNK_TILES, len(sub) - i)
                idx_t = sb.tile([128, CHUNK_TILES * 8], mybir.dt.int16,
                                tag=f"idx{tag}")
                nc.sync.dma_start(
                    idx_t[:, : k * 8], idx_dram[:, gi * 8 : (gi + k) * 8]
                )
                s_t = sp.tile([128, CHUNK_TILES * 128], cfg["sdt"], tag="s_t")
                nc.scalar.dma_start(
                    s_t[:, : k * 128], smat_dram[:, gi * 128 : (gi + k) * 128]
                )
                gat = sb.tile([128, CHUNK_TILES, GE], gdt, tag="gat")
                nc.gpsimd.dma_gather(
                    gat[:, :k, :], table(be), idx_t[:, : k * 8],
                    k * 128, k * 128, GE, queue_num=cfg["_q"] % 4,
                )
                cfg["_q"] += 1

                for tl in range(k):
                    b, t = sub[i + tl]
                    if t == 0:
                        cur_ps = ps_run.tile([DF, 128], f32, tag="runps")
                    nc.tensor.matmul(
                        cur_ps[:], gat[:, tl, :DF],
                        s_t[:, tl * 128 : (tl + 1) * 128],
                        start=(t == 0), stop=(t == int(T[be][b]) - 1),
                    )
                    if t == int(T[be][b]) - 1:
                        if b not in acc_tiles:
                            acc_tiles[b] = pools["accp"].tile(
                                [DF, 128], f32, tag=f"acc{b}", name=f"acc{tag}_{b}"
                            )
                            nc.vector.tensor_copy(acc_tiles[b][:], cur_ps[:])
                        else:
                            nc.vector.tensor_add(
                                acc_tiles[b][:], acc_tiles[b][:], cur_ps[:]
                            )
                        if be == last_beta[b]:
                            self_run_and_epilogue(b)
                i += k
                gi += k
        for b in blocks:
            if last_beta[b] < 0:
                self_run_and_epilogue(b)
        if group_cb is not None:
            group_cb()


def _build_program(T, wshapes, use_bf16):
    f32 = mybir.dt.float32
    gdt = mybir.dt.bfloat16 if use_bf16 else f32
    sdt = mybir.dt.float8e4 if (use_bf16 and S_FP8) else gdt
    bf16 = mybir.dt.bfloat16
    # gather table row elements: both layers 128 (L2 zero-padded) in bf16
    ge2 = 128 if use_bf16 else H1
    nc = bacc.Bacc("TRN2", num_swdge_queues=4)

    P = 128 * int(T.sum())

    xp_d = nc.dram_tensor("xperm", [SLOTS, D_IN], gdt, kind="ExternalInput")
    dcol_d = nc.dram_tensor("dinvcol", [128, BLKN], f32, kind="ExternalInput")
    dmat_d = nc.dram_tensor("dinvmat", [128, SLOTS], bf16, kind="ExternalInput")
    idx1_d = nc.dram_tensor("idx1", [128, P // 16], mybir.dt.int16, kind="ExternalInput")
    smat1_d = nc.dram_tensor("smat1", [128, P], sdt, kind="ExternalInput")
    idx2_d = nc.dram_tensor("idx2", [128, P // 16], mybir.dt.int16, kind="ExternalInput")
    smat2_d = nc.dram_tensor("smat2", [128, P], sdt, kind="ExternalInput")
    wdr = {}
    for name, shp in wshapes.items():
        wdr[name] = nc.dram_tensor(name, list(shp), f32, kind="ExternalInput")
    ident_d = nc.dram_tensor("ident", [128, 128], f32, kind="ExternalInput")
    identg_d = nc.dram_tensor("identg", [128, 128], sdt, kind="ExternalInput")
    y_d = nc.dram_tensor("y", [BLKN, 128], f32, kind="ExternalOutput")

    with tile.TileContext(nc) as tc:
        with (
            tc.tile_pool(name="cst", bufs=1) as cst,
            tc.tile_pool(name="sb", bufs=6) as sb,
            tc.tile_pool(name="sp", bufs=10) as sp,
            tc.tile_pool(name="accp", bufs=1) as accp,
            tc.tile_pool(name="hp", bufs=4) as hp,
            tc.tile_pool(name="ps_run", bufs=5, space="PSUM") as ps_run,
            tc.tile_pool(name="ps_epi", bufs=3, space="PSUM") as ps_epi,
            tc.tile_pool(name="dram", bufs=1, space="DRAM") as dram,
        ):
            pools = {"cst": cst, "sb": sb, "sp": sp, "accp": accp, "hp": hp,
                     "ps_run": ps_run, "ps_epi": ps_epi}

            ident_t = cst.tile([128, 128], f32)
            nc.sync.dma_start(ident_t[:], ident_d[:])
            identg_t = cst.tile([128, 128], sdt)
            nc.sync.dma_start(identg_t[:], identg_d[:])
            dcol_t = cst.tile([128, BLKN], f32)
            nc.sync.dma_start(dcol_t[:], dcol_d[:])
            dmat_t = cst.tile([128, SLOTS], bf16)
            nc.sync.dma_start(dmat_t[:], dmat_d[:])
            wt = {}
            for name in wshapes:
                wt[name] = cst.tile(list(wshapes[name]), f32, name=f"w_{name}")
                nc.sync.dma_start(wt[name][:], wdr[name][:])
            zero_t = None
            if use_bf16:
                zero_t = cst.tile([128, 128 - H1], gdt)
                nc.vector.memset(zero_t[:], 0.0)

            # per-quarter shard + gathered-table DRAM tiles
            xsc_sh = [dram.tile([QSLOT[g], D_IN], gdt, name=f"xsc{g}")
                      for g in range(4)]
            xt_full = [dram.tile([QROWS[g], D_IN], gdt, addr_space="Shared",
                                 name=f"xt{g}") for g in range(4)]
            h1_sh = [dram.tile([QSLOT[g], ge2], gdt, name=f"h1s{g}")
                     for g in range(4)]
            h1t_full = [dram.tile([QROWS[g], ge2], gdt, addr_space="Shared",
                                  name=f"h1t{g}") for g in range(4)]

            def ag(ins_t, outs_t):
                nc.gpsimd.collective_compute(
                    "AllGather",
                    mybir.AluOpType.bypass,
                    ins=[ins_t.opt()],
                    outs=[outs_t.opt()],
                    replica_groups=[list(range(NCORES))],
                )

            # ---- scale pass: xsc = dinv[node] * xperm, quarter-pipelined
            for g in range(4):
                for bl in range(QBLO[g], QBLO[g + 1]):
                    lb = bl - QBLO[g]
                    xpb = sb.tile([128, D_IN], gdt, tag="xpb")
                    nc.sync.dma_start(xpb[:], xp_d[bl * 128 : (bl + 1) * 128, :])
                    xsb = sb.tile([128, D_IN], gdt, tag="xsb")
                    nc.vector.tensor_scalar(
                        xsb[:], xpb[:], dcol_t[:, bl : bl + 1], None,
                        mybir.AluOpType.mult,
                    )
                    nc.scalar.dma_start(
                        xsc_sh[g][lb * 128 : (lb + 1) * 128, :], xsb[:]
                    )
                ag(xsc_sh[g], xt_full[g])

            def self_rows1(b):
                g = Q_OF_BLOCK[b]
                lb = b - QBLO[g]
                return xsc_sh[g][lb * 128 : (lb + 1) * 128, :]

            def self_rows2(b):
                g = Q_OF_BLOCK[b]
                lb = b - QBLO[g]
                return h1_sh[g][lb * 128 : (lb + 1) * 128, :H1]

            def epi1(b, acc_t):
                eps = ps_epi.tile([H1, 128], f32, tag="eps")
                nc.tensor.matmul(eps[:], wt["cW0"][:], acc_t[:], start=True, stop=True)
                dvb = dmat_t[:H1, b * 128 : (b + 1) * 128]
                ep2 = hp.tile([H1, 128], f32, tag="ep2")
                nc.vector.tensor_tensor(
                    ep2[:], eps[:], dvb, mybir.AluOpType.mult
                )
                h1T = hp.tile([H1, 128], f32, tag="h1T")
                nc.scalar.activation(
                    h1T[:], ep2[:], mybir.ActivationFunctionType.Tanh,
                    bias=wt["cb0"][:, 0:1],
                )
                h1s = hp.tile([H1, 128], f32, tag="h1s")
                nc.vector.tensor_tensor(
                    h1s[:], h1T[:], dvb, mybir.AluOpType.mult
                )
                tp = ps_epi.tile([128, H1], f32, tag="eps")
                nc.tensor.transpose(tp[:], h1s[:], ident_t[:H1, :H1])
                h1n = hp.tile([128, H1], gdt, tag="h1n")
                nc.vector.tensor_copy(h1n[:], tp[:])
                g = Q_OF_BLOCK[b]
                lb = b - QBLO[g]
                nc.scalar.dma_start(
                    h1_sh[g][lb * 128 : (lb + 1) * 128, :H1], h1n[:]
                )
                if use_bf16:
                    nc.sync.dma_start(
                        h1_sh[g][lb * 128 : (lb + 1) * 128, H1:], zero_t[:]
                    )

            groups1 = []
            for g in range(4):
                cb = (lambda gg: (lambda: ag(h1_sh[gg], h1t_full[gg])))(g)
                groups1.append((list(range(QBLO[g], QBLO[g + 1])), cb))

            _emit_conv(nc, pools, {
                "T": T, "feat": D_IN, "gelem": D_IN, "gdt": gdt, "sdt": sdt,
                "tag": "1",
                "table": lambda be: xt_full[be][:, :],
                "self_rows": self_rows1,
                "idx": idx1_d, "smat": smat1_d,
                "identg_t": identg_t,
                "epilogue": epi1,
                "groups": groups1,
            })

            def epi2(b, acc_t):
                e1 = ps_epi.tile([H1, 128], f32, tag="eps")
                nc.tensor.matmul(e1[:], wt["cW1"][:], acc_t[:], start=True, stop=True)
                dvb = dmat_t[:H1, b * 128 : (b + 1) * 128]
                e1s = hp.tile([H1, 128], f32, tag="e1s")
                nc.vector.tensor_tensor(
                    e1s[:], e1[:], dvb, mybir.AluOpType.mult
                )
                h2T = hp.tile([H1, 128], f32, tag="h2T")
                nc.scalar.activation(
                    h2T[:], e1s[:], mybir.ActivationFunctionType.Tanh,
                    bias=wt["cb1"][:, 0:1],
                )
                e2 = ps_epi.tile([H1, 128], f32, tag="eps")
                nc.tensor.matmul(e2[:], wt["fW0"][:], h2T[:], start=True, stop=True)
                h3T = hp.tile([H1, 128], f32, tag="h3T")
                nc.scalar.activation(
                    h3T[:], e2[:], mybir.ActivationFunctionType.Tanh,
                    bias=wt["fb0"][:, 0:1],
                )
                e3 = ps_epi.tile([32, 128], f32, tag="eps")
                nc.tensor.matmul(e3[:], wt["fW1"][:], h3T[:], start=True, stop=True)
                h4T = hp.tile([32, 128], f32, tag="h4T")
                nc.scalar.activation(
                    h4T[:], e3[:], mybir.ActivationFunctionType.Tanh,
                    bias=wt["fb1"][:, 0:1],
                )
                e4 = ps_epi.tile([1, 128], f32, tag="eps")
                nc.tensor.matmul(e4[:], wt["fW2"][:], h4T[:], start=True, stop=True)
                yrow = hp.tile([1, 128], f32, tag="yrow")
                nc.vector.tensor_scalar_add(yrow[:], e4[:], wt["fb2"][0:1, 0:1])
                nc.sync.dma_start(y_d[b : b + 1, :], yrow[:])

            _emit_conv(nc, pools, {
                "T": T, "feat": H1, "gelem": ge2, "gdt": gdt, "sdt": sdt,
                "tag": "2",
                "table": lambda be: h1t_full[be][:, :],
                "self_rows": self_rows2,
                "idx": idx2_d, "smat": smat2_d,
                "identg_t": identg_t,
                "epilogue": epi2,
                "groups": [(list(range(BLKN)), None)],
            })

    nc.compile()
    return nc


# ------------------------------------------------------------------- driver
def kernel(**inputs):
    global LAST_EXEC_TIME_NS, LAST_RESULTS
    use_bf16 = os.environ.get("BASS_GCN_BF16", "1") == "1"
    np_gdt = ml_dtypes.bfloat16 if use_bf16 else np.float32

    x = np.ascontiguousarray(np.asarray(inputs["x"], np.float32))
    ei = np.asarray(inputs["edge_index"], np.int64)
    src, dst = ei[0], ei[1]

    weights = {
        "cW0": np.ascontiguousarray(np.asarray(inputs["cW0"], np.float32)),
        "cb0": np.asarray(inputs["cb0"], np.float32).reshape(H1, 1),
        "cW1": np.ascontiguousarray(np.asarray(inputs["cW1"], np.float32)),
        "cb1": np.asarray(inputs["cb1"], np.float32).reshape(H1, 1),
        "fW0": np.ascontiguousarray(np.asarray(inputs["fW0"], np.float32)),
        "fb0": np.asarray(inputs["fb0"], np.float32).reshape(H1, 1),
        "fW1": np.ascontiguousarray(np.asarray(inputs["fW1"], np.float32)),
        "fb1": np.asarray(inputs["fb1"], np.float32).reshape(32, 1),
        "fW2": np.ascontiguousarray(np.asarray(inputs["fW2"], np.float32)),
        "fb2": np.asarray(inputs["fb2"], np.float32).reshape(1, 1),
    }

    np_sdt = ml_dtypes.float8_e4m3fn if (use_bf16 and S_FP8) else np_gdt
    streams, xperms, dinvcols, dinvmats, T, node_of_pos = _plan(src, dst, x, np_sdt)

    nc = _build_program(T, {k: v.shape for k, v in weights.items()}, use_bf16)

    ident = np.eye(128, dtype=np.float32)
    identg = np.ascontiguousarray(ident.astype(np_sdt))

    in_maps = []
    for c in range(NCORES):
        idx1, smat1, idx2, smat2 = streams[c]
        m = {"xperm": np.ascontiguousarray(xperms[c].astype(np_gdt)),
             "dinvcol": dinvcols[c], "dinvmat": dinvmats[c],
             "idx1": idx1, "smat1": smat1, "idx2": idx2, "smat2": smat2,
             "ident": ident, "identg": identg}
        m.update(weights)
        in_maps.append(m)

    trace = os.environ.get("BASS_GCN_TRACE") == "1"
    res = run_bass_kernel_spmd(nc, in_maps, list(range(NCORES)), trace=trace)
    if trace:
        LAST_EXEC_TIME_NS = res.exec_time_ns
    LAST_RESULTS = res

    out = np.zeros((N, 1), np.float32)
    for c in range(NCORES):
        yflat = res.results[c]["y"].reshape(SLOTS)
        valid = node_of_pos[c] >= 0
        out[node_of_pos[c][valid], 0] = yflat[valid]
    return out
